# revision 28
# baseline (speedup 1.0000x reference)
"""Trainium2 Bass kernel for nn_CTRPredictor (gnn_message_passing), v3.

score[e] = dot(normalize(x[src[e]]), normalize(x[dst[e]]))  for E edges.

Strategy (8 NeuronCores, SPMD):
  - Cores 2a, 2a+1 split the edges whose src lies in pair-bank a
    (nodes [25000a, 25000(a+1))), so each core's src rows come from one
    6.4MB bf16 table that fits in SBUF.
  - The src side is "gathered" by the idle TensorEngine: edges are packed
    into 128-slot batches whose src rows fit a 3-block window of a shared
    (core-uniform) block schedule; 3 accumulating one-hot matmuls per batch
    select the rows from the SBUF table (no DMA descriptors at all).
  - The dst side remains a SWDGE dma_gather from the DRAM-replicated table
    (int16 ids within 25088-row apertures), now the only descriptor load on
    the 4 queues (half of the baseline's).
  - Normalized x travels in a row-of-block-on-partition transposed layout:
    a pair AllGather yields the SBUF src table, two half AllGathers build
    the dst apertures, each overlapping gathers on earlier banks.
  - DVE builds one-hots (is_equal vs iota) and does the bf16 mult+reduce;
    ACT drains PSUM to SBUF.
"""

import numpy as np

N = 100000
D = 128
E = 640000
CORES = 8
SLICE = 12500             # nodes normalized per core
PBANK = 25000             # nodes per src pair-bank (2 slices)
SBLK = 98                 # 128-row vblocks per slice (98*128 = 12544)
SLOTPAD = SBLK * 128      # padded slice rows (44 pad rows of ones)
NVB = 2 * SBLK            # vblocks per pair-bank table
HBLK = 49                 # vblocks per half AllGather
APER = 4 * 128 * HBLK     # rows per dst aperture (25088, int16-safe)
NDG = 4                   # dst groups: (half h, rank-half rh)
GCALL = 2560              # slots per dma_gather call (20 batches)
WIN = 2                   # src block window per batch

_CACHE = {}
LAST_RESULTS = None
RUN_KWARGS = {}


def _call_caps(cap):
    calls = []
    while cap > 0:
        c = min(GCALL, cap)
        calls.append(c)
        cap -= c
    return calls


def _build(scheds):
    """scheds: tuple of 4 tuples, per-dst-group batch base-block schedule."""
    from concourse import bass, bacc, tile, mybir

    f32 = mybir.dt.float32
    bf16 = mybir.dt.bfloat16
    i16 = mybir.dt.int16

    slots_dg = [len(s) * 128 for s in scheds]
    slots_total = sum(slots_dg)
    icols_total = slots_total // 16
    scol_total = slots_total // 128

    nc = bacc.Bacc("TRN2", target_bir_lowering=False, debug=False,
                   num_devices=CORES, num_swdge_queues=4,
                   dynamic_dma_scratch_size=40960)

    xsl_d = nc.dram_tensor("xsl", [128, SLOTPAD], f32, kind="ExternalInput")
    ids_d = nc.dram_tensor("ids", [128, slots_total], bf16,
                           kind="ExternalInput")
    didx_d = nc.dram_tensor("dst_idx", [128, icols_total], i16,
                            kind="ExternalInput")
    tidx_d = nc.dram_tensor("tbl_idx", [128, 112], i16,
                            kind="ExternalInput")
    out_d = nc.dram_tensor("out", [128, scol_total], f32,
                           kind="ExternalOutput")

    with tile.TileContext(nc) as tc:
        with tc.tile_pool(name="dram", bufs=1, space="DRAM") as dp, \
             tc.tile_pool(name="persist", bufs=1) as pp:

            didx = pp.tile([128, icols_total], i16)
            tidx = pp.tile([128, 112], i16)
            score = pp.tile([128, scol_total], f32)
            tbl = pp.tile([128, NVB * D], bf16)
            iot = pp.tile([128, WIN], bf16)
            nc.gpsimd.iota(out=iot[:, :], pattern=[[128, WIN]], base=0,
                           channel_multiplier=1,
                           allow_small_or_imprecise_dtypes=True)

            # ---- normalize this core's slice (transposed vblock layout);
            # the normalized own slice IS the first half of the SBUF table
            ags = []
            with tc.tile_pool(name="ph0", bufs=1) as p0, \
                 tc.tile_pool(name="xlp", bufs=2) as xlp, \
                 tc.tile_pool(name="sqp", bufs=2) as sqp:
                ns = p0.tile([128, SBLK], f32)
                nrm = p0.tile([128, SBLK], f32)
                rns = p0.tile([128, SBLK], f32)
                bounds = [0, 13, 25, 37, HBLK, 62, 74, 86, SBLK]
                for ci in range(8):
                    b0, b1 = bounds[ci], bounds[ci + 1]
                    csl = slice(b0 * D, b1 * D)
                    nsl = slice(b0, b1)
                    nb = b1 - b0
                    xsl = xlp.tile([128, 13 * D], f32, tag="xl")
                    nc.sync.dma_start(out=xsl[:, :nb * D],
                                      in_=xsl_d.ap()[:, csl])
                    sq = sqp.tile([128, 13 * D], f32, tag="sq")
                    nc.scalar.activation(
                        out=sq[:, :nb * D], in_=xsl[:, :nb * D],
                        func=mybir.ActivationFunctionType.Square)
                    nc.vector.tensor_reduce(
                        out=ns[:, nsl],
                        in_=sq[:, :nb * D].rearrange("p (r d) -> p r d", d=D),
                        axis=mybir.AxisListType.X,
                        op=mybir.AluOpType.add,
                    )
                    nc.scalar.activation(
                        out=nrm[:, nsl], in_=ns[:, nsl],
                        func=mybir.ActivationFunctionType.Sqrt)
                    nc.vector.reciprocal(out=rns[:, nsl], in_=nrm[:, nsl])
                    nc.vector.tensor_mul(
                        out=tbl[:, csl].rearrange("p (r d) -> p r d", d=D),
                        in0=xsl[:, :nb * D].rearrange("p (r d) -> p r d",
                                                      d=D),
                        in1=rns[:, nsl].unsqueeze(-1).to_broadcast(
                            [128, nb, D]),
                    )
                    if ci == 3:   # blocks [0, 49) done -> first half AG
                        agin = dp.tile([128, HBLK * D], bf16, name="agh0")
                        htab = dp.tile([1024, HBLK * D], bf16, name="htab0",
                                       addr_space="Shared")
                        nc.sync.dma_start(out=agin[:, :],
                                          in_=tbl[:, :HBLK * D])
                        nc.gpsimd.collective_compute(
                            "AllGather", mybir.AluOpType.bypass,
                            replica_groups=[list(range(CORES))],
                            ins=[agin.opt()], outs=[htab.opt()],
                        )
                        ags.append(htab)
                    if ci == 7:
                        agin1 = dp.tile([128, HBLK * D], bf16, name="agh1")
                        htab1 = dp.tile([1024, HBLK * D], bf16, name="htab1",
                                        addr_space="Shared")
                        nc.sync.dma_start(out=agin1[:, :],
                                          in_=tbl[:, HBLK * D:SLOTPAD])
                        nc.gpsimd.collective_compute(
                            "AllGather", mybir.AluOpType.bypass,
                            replica_groups=[list(range(CORES))],
                            ins=[agin1.opt()], outs=[htab1.opt()],
                        )
                        ags.append(htab1)
                        # index loads ride behind the xsl chunks
                        nc.sync.dma_start(out=didx[:, :], in_=didx_d.ap())
                        nc.sync.dma_start(out=tidx[:, :], in_=tidx_d.ap())

            def _extract_partner(h):
                # partner slice: pull this half's 49 blocks from the half
                # AG into tbl's second half (idx is per-core data)
                nc.gpsimd.dma_gather(
                    out_ap=tbl[:, (SBLK + HBLK * h) * D:
                               (SBLK + HBLK * (h + 1)) * D].rearrange(
                        "p (g e) -> p g e", e=896),
                    in_ap=ags[h][:, :].rearrange("q (g e) -> (q g) e", e=896),
                    idxs_ap=tidx[:, 56 * h:56 * (h + 1)],
                    num_idxs=896, num_idxs_reg=896, elem_size=896,
                    single_packet=False, queue_num=3,
                )

            # ---- main loop over dst groups / calls ----
            with tc.tile_pool(name="xdp", bufs=8) as xdp, \
                 tc.tile_pool(name="xsp", bufs=3) as xsp, \
                 tc.tile_pool(name="ohp", bufs=2) as ohp, \
                 tc.tile_pool(name="idp", bufs=3) as idp, \
                 tc.tile_pool(name="psp", bufs=6, space="PSUM") as psp:
                qn = 0
                # static offsets for every (dg, call)
                call_info = {}
                icol_off = scol_off = slot_off = 0
                for dg in range(NDG):
                    t_base = 0
                    for k, cap in enumerate(_call_caps(slots_dg[dg])):
                        call_info[(dg, k)] = (cap, icol_off, scol_off,
                                              slot_off, t_base)
                        icol_off += cap // 16
                        scol_off += cap // 128
                        slot_off += cap
                        t_base += cap // 128
                # emission order: dg0/dg1 calls whose src blocks stay below
                # the partner-h1 region first, then the h1 extraction, then
                # dg2/dg3, then the held-back tails
                first_h1 = SBLK + HBLK
                order_emit = []
                tails = []
                for dg in (0, 1):
                    sched = scheds[dg]
                    ncalls = len(_call_caps(slots_dg[dg]))
                    for k in range(ncalls):
                        t0 = k * (GCALL // 128)
                        t1 = min(t0 + GCALL // 128, len(sched))
                        if max(sched[t0:t1]) + WIN - 1 >= first_h1:
                            tails.append((dg, k))
                        else:
                            order_emit.append((dg, k))
                order_emit.append(("x1", 0))
                for dg in (2, 3):
                    for k in range(len(_call_caps(slots_dg[dg]))):
                        order_emit.append((dg, k))
                order_emit.extend(tails)

                _extract_partner(0)
                for dg, k in order_emit:
                    if dg == "x1":
                        _extract_partner(1)
                        continue
                    h, rh = dg // 2, dg % 2
                    aper = ags[h][:, :].rearrange(
                        "q (j f) -> (q j) f", f=D)[rh * APER:(rh + 1) * APER,
                                                   :]
                    sched = scheds[dg]
                    for cap, icol_off, scol_off, slot_off, t_base in \
                            [call_info[(dg, k)]]:
                        cc = cap // 128
                        ic = cap // 16
                        xd = xdp.tile([128, GCALL], bf16, tag="xd")
                        nc.gpsimd.dma_gather(
                            out_ap=xd[:, :cap].rearrange(
                                "p (c d) -> p c d", d=D),
                            in_ap=aper,
                            idxs_ap=didx[:, icol_off:icol_off + ic],
                            num_idxs=cap, num_idxs_reg=cap, elem_size=D,
                            single_packet=False, queue_num=qn % 4,
                        )
                        qn += 1
                        ids_t = idp.tile([128, GCALL], bf16, tag="ids")
                        nc.scalar.dma_start(
                            out=ids_t[:, :cap],
                            in_=ids_d.ap()[:, slot_off:slot_off + cap])
                        ohs = []
                        for k in range(WIN):
                            oh = ohp.tile([128, GCALL], bf16, tag=f"oh{k}")
                            nc.vector.tensor_tensor(
                                out=oh[:, :cap],
                                in0=ids_t[:, :cap],
                                in1=iot[:, k:k + 1].to_broadcast([128, cap]),
                                op=mybir.AluOpType.is_equal,
                            )
                            ohs.append(oh)
                        xs = xsp.tile([128, GCALL], bf16, tag="xs")
                        for q in range(cap // 512):
                            ps = psp.tile([128, 512], f32, tag="ps")
                            for b in range(4):
                                t = t_base + q * 4 + b
                                j0 = sched[t]
                                col = (q * 4 + b) * 128
                                for k in range(WIN):
                                    nc.tensor.matmul(
                                        out=ps[:, b * 128:(b + 1) * 128],
                                        lhsT=ohs[k][:, col:col + 128],
                                        rhs=tbl[:, (j0 + k) * D:
                                                (j0 + k + 1) * D],
                                        start=(k == 0), stop=(k == WIN - 1),
                                    )
                            nc.scalar.activation(
                                out=xs[:, q * 512:(q + 1) * 512],
                                in_=ps[:, :],
                                func=mybir.ActivationFunctionType.Copy)
                        nc.vector.tensor_mul(out=ohs[0][:, :cap],
                                             in0=xs[:, :cap],
                                             in1=xd[:, :cap])
                        nc.vector.tensor_reduce(
                            out=score[:, scol_off:scol_off + cc],
                            in_=ohs[0][:, :cap].rearrange("p (c d) -> p c d",
                                                          d=D),
                            axis=mybir.AxisListType.X,
                            op=mybir.AluOpType.add,
                        )

                nc.sync.dma_start(out=out_d.ap(), in_=score[:, :])

    nc.compile()
    return nc


def _src_map(s):
    """src node -> (pair-bank, owner rank within pair, slice-vblock, row).

    The per-core table holds the core's OWN slice first, then the
    partner's, so the final vblock is 98*(owner != core) + j_sl.
    """
    a = s // PBANK
    w = s - a * PBANK
    rank = w // SLICE
    ws = w - rank * SLICE
    return a, rank, ws // 128, ws % 128


def _dst_map(n):
    """dst node -> (dst group, aperture-local index)."""
    r = n // SLICE
    w = n - r * SLICE
    j = w // 128
    p = w % 128
    h = (j >= HBLK).astype(np.int64)
    rh = (r >= 4).astype(np.int64)
    local = ((r % 4) * 128 + p) * HBLK + (j - HBLK * h)
    return 2 * h + rh, local


def _wrap_idx(flat):
    blk = flat.reshape(-1, 16).T
    return np.tile(blk, (8, 1))


def _build_scheds(counts, inflate=0, use_mean=False):
    """counts: [8, NDG, NVB] per-core block counts -> uniform schedules."""
    if use_mean:
        caps = np.ceil(counts.mean(axis=0) + 1.0).astype(np.int64) + inflate
    else:
        caps = counts.max(axis=0) + inflate  # [NDG, NVB]
    scheds = []
    for dg in range(NDG):
        rem = caps[dg].astype(np.int64).copy()
        sched = []
        total = int(rem.sum())
        j = 0
        while total > 0:
            while j < NVB - 1 and rem[j] == 0:
                j += 1
            j0 = min(j, NVB - WIN)
            room = 128
            jj = j0
            while room > 0 and jj <= min(j0 + WIN - 1, NVB - 1):
                take = int(min(room, rem[jj]))
                rem[jj] -= take
                room -= take
                total -= take
                if rem[jj] == 0:
                    jj += 1
            sched.append(j0)
        while len(sched) % 4:
            sched.append(NVB - WIN)
        scheds.append(tuple(sched))
    return tuple(scheds)


def _assign_core(edges, scheds):
    """edges: list per dg of (eid, j, p, dl) arrays. Returns slot data."""
    slots_dg = [len(s) * 128 for s in scheds]
    slots_total = sum(slots_dg)
    ids_flat = np.full(slots_total, -1, dtype=np.int16)
    didx_flat = np.zeros(slots_total, dtype=np.int16)
    rows = np.empty(E // CORES + 4096, dtype=np.int64)
    cols = np.empty(E // CORES + 4096, dtype=np.int64)
    n_edges = 0
    slot_off = 0
    t_off = 0
    for dg in range(NDG):
        eid, jb, pr, dl = edges[dg]
        sched = np.asarray(scheds[dg])
        order = np.argsort(jb, kind="stable")
        eid, jb, pr, dl = eid[order], jb[order], pr[order], dl[order]
        # greedy: batches in order take earliest-block remaining edges
        ptr = 0
        nE = eid.size
        for t, j0 in enumerate(sched):
            room = 128
            s0 = slot_off + t * 128
            while room > 0 and ptr < nE and jb[ptr] < j0 + WIN:
                if jb[ptr] < j0:
                    raise RuntimeError("scheduler stranded an edge")
                s = s0 + (128 - room)
                ids_flat[s] = 128 * (jb[ptr] - j0) + pr[ptr]
                didx_flat[s] = dl[ptr]
                e = eid[ptr]
                rows[e] = s % 128
                cols[e] = t_off + t
                room -= 1
                ptr += 1
            if ptr >= nE and room == 128:
                pass  # trailing pad batches
        if ptr < nE:
            raise RuntimeError(f"dg{dg}: {nE - ptr} edges unplaced")
        n_edges += nE
        slot_off += slots_dg[dg]
        t_off += slots_dg[dg] // 128
    return ids_flat, didx_flat, rows, cols, n_edges


def kernel(x, src, dst):
    global LAST_RESULTS
    from concourse.bass_utils import run_bass_kernel_spmd

    x32 = np.ascontiguousarray(np.asarray(x, dtype=np.float32))
    src_i = np.asarray(src).astype(np.int64)
    dst_i = np.asarray(dst).astype(np.int64)

    a_all, rank_all, jsl_all, pr_all = _src_map(src_i)
    dg_all, dl_all = _dst_map(dst_i)

    # edge -> core: pair-bank a holds cores 2a, 2a+1; alternate within
    # each (a, rank, dg, block) bucket so per-core block counts balance
    key = ((a_all * 2 + rank_all) * NDG + dg_all) * SBLK + jsl_all
    order = np.argsort(key, kind="stable")
    half = np.empty(E, dtype=np.int64)
    sizes = np.bincount(key, minlength=8 * NDG * SBLK)
    off = 0
    for kk in range(8 * NDG * SBLK):
        ids_b = order[off:off + sizes[kk]]
        off += sizes[kk]
        half[ids_b] = np.arange(ids_b.size) % 2
    core_all = 2 * a_all + half

    # per-core, per-dg edge arrays + block counts; vblock j is
    # core-relative: own slice first (98*(owner != core) + j_sl)
    counts = np.zeros((CORES, NDG, NVB), dtype=np.int64)
    core_edges = []
    eid_local = np.empty(E, dtype=np.int64)
    for c in range(CORES):
        sel = np.where(core_all == c)[0]
        eid_local[sel] = np.arange(sel.size)
        jb_c = SBLK * (rank_all[sel] != (c % 2)) + jsl_all[sel]
        per_dg = []
        for dg in range(NDG):
            mm = dg_all[sel] == dg
            m = sel[mm]
            per_dg.append((eid_local[m], jb_c[mm], pr_all[m], dl_all[m]))
            counts[c, dg] = np.bincount(jb_c[mm], minlength=NVB)
        core_edges.append((sel, per_dg))

    scheds = None
    for use_mean, inflate in ((True, 0), (True, 1), (True, 3),
                              (False, 0), (False, 2), (False, 8),
                              (False, 32)):
        cand = _build_scheds(counts, inflate, use_mean)
        try:
            for c in range(CORES):
                _assign_core(core_edges[c][1], cand)
        except RuntimeError:
            continue
        scheds = cand
        break
    assert scheds is not None, "schedule infeasible even with inflation"

    if scheds not in _CACHE:
        _CACHE[scheds] = _build(scheds)
    nc = _CACHE[scheds]

    slots_dg = [len(s) * 128 for s in scheds]
    slots_total = sum(slots_dg)

    in_maps = []
    inv = []
    for c in range(CORES):
        sel, per_dg = core_edges[c]
        ids_flat, didx_flat, rows, cols, nE = _assign_core(per_dg, scheds)
        assert nE == sel.size
        inv.append((sel, rows, cols))

        # didx wrap per call
        icols_total = slots_total // 16
        didx = np.zeros((128, icols_total), dtype=np.int16)
        icol_off = 0
        pos = 0
        for dg in range(NDG):
            for cap in _call_caps(slots_dg[dg]):
                ic = cap // 16
                didx[:, icol_off:icol_off + ic] = _wrap_idx(
                    didx_flat[pos:pos + cap])
                icol_off += ic
                pos += cap

        # xsl: slice in transposed vblock layout, pad rows = 1.0
        xs = np.ones((SLOTPAD, D), dtype=np.float32)
        xs[:SLICE] = x32[c * SLICE:(c + 1) * SLICE]
        xsl = np.ascontiguousarray(
            xs.reshape(SBLK, 128, D).transpose(1, 0, 2).reshape(
                128, SLOTPAD))

        # partner-slice extraction gather indices (same for both halves)
        rp = c ^ 1
        i_arr = np.arange(896)
        tvals = ((rp * 128 + i_arr % 128) * 7 + i_arr // 128).astype(
            np.int16)
        tidx = np.concatenate([_wrap_idx(tvals), _wrap_idx(tvals)], axis=1)

        import ml_dtypes
        ids_bf = ids_flat.astype(np.float32).astype(ml_dtypes.bfloat16)

        in_maps.append({
            "xsl": xsl,
            "ids": np.ascontiguousarray(np.tile(ids_bf[None, :], (128, 1))),
            "dst_idx": np.ascontiguousarray(didx),
            "tbl_idx": np.ascontiguousarray(tidx),
        })

    res = run_bass_kernel_spmd(nc, in_maps, core_ids=list(range(CORES)),
                               **RUN_KWARGS)
    LAST_RESULTS = res

    out = np.empty(E, dtype=np.float32)
    for c in range(CORES):
        tilev = np.asarray(res.results[c]["out"])
        sel, rows, cols = inv[c]
        nE = sel.size
        out[sel] = tilev[rows[:nE], cols[:nE]]
    return out.reshape(E, 1)


# revision 29
# speedup vs baseline: 1.0320x; 1.0320x over previous
"""Trainium2 Bass kernel for nn_CTRPredictor (gnn_message_passing).

score[e] = dot(normalize(x[src[e]]), normalize(x[dst[e]]))  for E edges.

Strategy (8 NeuronCores, SPMD), v2:
  - Edges sharded: core i gets edges [i*80000, (i+1)*80000).
  - Each core L2-normalizes its 12500-node slice of x in 4 pipelined
    quarter-chunks ([125, 100*128] layout so每 chunk uses 125 partitions),
    emitting each normalized quarter to a quarter AllGather as soon as it
    is ready; AG chunk q produces table bank q (25000 rows, bf16) so
    gathers on early banks overlap later collectives.
  - Host groups each core's edges by (src_bank, dst_bank) into 16 groups
    (bank-local indices fit dma_gather's int16) with JIT-exact capacities
    (max over cores, rounded to 128) instead of a fixed padded capacity.
  - Per group: dma_gather x_norm[src] and x_norm[dst] rows (256B bf16)
    across 4 SWDGE queues, DVE bf16 multiply + grouped reduce -> scores.
  - Host un-permutes scores back to edge order.
"""

import numpy as np

N = 100000
D = 128
E = 640000
CORES = 8
EPC = E // CORES          # 80000 edges per core
SLICE = N // CORES        # 12500 nodes normalized per core
QSL = SLICE // 4          # 3125-row quarter slices (AllGather chunks)
NBANK = 4
BANK = N // NBANK         # 25000 rows per bank (= one AG chunk output)
NGRP = NBANK * NBANK      # 16 (src_bank, dst_bank) groups
GCALL = 2688              # max indices per dma_gather call

# groups in bank-availability order: group (a,b) is gatherable once
# AllGather max(a,b) has completed
GROUP_ORDER = sorted(range(NGRP),
                     key=lambda g: (max(g // NBANK, g % NBANK),
                                    g // NBANK, g % NBANK))

_CACHE = {}
LAST_RESULTS = None
RUN_KWARGS = {}  # extra kwargs for run_bass_kernel_spmd (used by test harness)


def _call_caps(cap):
    """Split a group capacity (multiple of 128) into dma_gather call sizes."""
    calls = []
    while cap > 0:
        c = min(GCALL, cap)
        calls.append(c)
        cap -= c
    return calls


def _build(caps):
    """caps: tuple of 16 per-group capacities (each a multiple of 128)."""
    from concourse import bass, bacc, tile, mybir

    f32 = mybir.dt.float32
    bf16 = mybir.dt.bfloat16
    i16 = mybir.dt.int16

    icols_total = sum(caps) // 16
    scol_total = sum(caps) // 128

    nc = bacc.Bacc("TRN2", target_bir_lowering=False, debug=False,
                   num_devices=CORES, num_swdge_queues=4,
                   dynamic_dma_scratch_size=81920)

    # node slice in [125, 100*128] layout: slice-local node n with
    # q=n//3125, w=n%3125 sits at partition w//25, col q*25 + w%25.
    xsl_d = nc.dram_tensor("xsl", [125, 100 * D], f32, kind="ExternalInput")
    sidx_d = nc.dram_tensor("src_idx", [128, icols_total], i16,
                            kind="ExternalInput")
    didx_d = nc.dram_tensor("dst_idx", [128, icols_total], i16,
                            kind="ExternalInput")
    out_d = nc.dram_tensor("out", [128, scol_total], f32,
                           kind="ExternalOutput")

    CCH = 25 * D  # free-dim span of one quarter chunk [125, 25*128]

    with tile.TileContext(nc) as tc:
        with tc.tile_pool(name="dram", bufs=1, space="DRAM") as dp, \
             tc.tile_pool(name="persist", bufs=1) as pp:

            # ---- index tables + score accumulator ----
            sidx = pp.tile([128, icols_total], i16)
            didx = pp.tile([128, icols_total], i16)
            nc.sync.dma_start(out=sidx[:, :], in_=sidx_d.ap())
            nc.sync.dma_start(out=didx[:, :], in_=didx_d.ap())
            score = pp.tile([128, scol_total], f32)

            # ---- phase 0: normalize this core's slice in 4 chunks ----
            banks = []
            with tc.tile_pool(name="ph0", bufs=1) as p0, \
                 tc.tile_pool(name="xlp", bufs=2) as xlp, \
                 tc.tile_pool(name="sqp", bufs=2) as sqp:
                ntile = p0.tile([125, 100 * D], bf16)
                ns = p0.tile([125, 100], f32)
                nrm = p0.tile([125, 100], f32)
                rns = p0.tile([125, 100], f32)
                bounds = [0, 13, 25, 38, 50, 63, 75, 88, 100]
                for ci in range(8):
                    b0, b1 = bounds[ci], bounds[ci + 1]
                    nb = b1 - b0
                    csl = slice(b0 * D, b1 * D)
                    nsl = slice(b0, b1)
                    xsl = xlp.tile([125, 13 * D], f32, tag="xl")
                    nc.sync.dma_start(out=xsl[:, :nb * D],
                                      in_=xsl_d.ap()[:, csl])
                    sq = sqp.tile([125, 13 * D], f32, tag="sq")
                    nc.scalar.activation(
                        out=sq[:, :nb * D], in_=xsl[:, :nb * D],
                        func=mybir.ActivationFunctionType.Square)
                    nc.vector.tensor_reduce(
                        out=ns[:, nsl],
                        in_=sq[:, :nb * D].rearrange("p (r d) -> p r d", d=D),
                        axis=mybir.AxisListType.X,
                        op=mybir.AluOpType.add,
                    )
                    nc.scalar.activation(
                        out=nrm[:, nsl], in_=ns[:, nsl],
                        func=mybir.ActivationFunctionType.Sqrt)
                    nc.vector.reciprocal(out=rns[:, nsl], in_=nrm[:, nsl])
                    nc.vector.tensor_mul(
                        out=ntile[:, csl].rearrange("p (r d) -> p r d", d=D),
                        in0=xsl[:, :nb * D].rearrange("p (r d) -> p r d",
                                                      d=D),
                        in1=rns[:, nsl].unsqueeze(-1).to_broadcast(
                            [125, nb, D]),
                    )
                    if ci % 2 == 0:
                        continue
                    q = ci // 2
                    csl = slice(q * CCH, (q + 1) * CCH)
                    # quarter AllGather: output is table bank q
                    agin = dp.tile([QSL, D], bf16, name=f"agin{q}")
                    htab = dp.tile([BANK, D], bf16, name=f"htab{q}",
                                   addr_space="Shared")
                    nc.sync.dma_start(
                        out=agin[:, :].rearrange("(p r) d -> p (r d)", p=125),
                        in_=ntile[:, csl],
                    )
                    nc.gpsimd.collective_compute(
                        "AllGather",
                        mybir.AluOpType.bypass,
                        replica_groups=[list(range(CORES))],
                        ins=[agin.opt()],
                        outs=[htab.opt()],
                    )
                    banks.append(htab)

            # ---- main loop: gathers on 4 queues, DVE dot per call ----
            with tc.tile_pool(name="ga", bufs=5) as ga, \
                 tc.tile_pool(name="gb", bufs=5) as gb:
                qn = 0
                icol_off = 0
                scol_off = 0
                for g in GROUP_ORDER:
                    ba, bb = g // NBANK, g % NBANK
                    for cap in _call_caps(caps[g]):
                        cc = cap // 128   # gathered row-columns this call
                        ic = cap // 16    # index columns this call
                        xs_t = ga.tile([128, (GCALL // 128) * D], bf16,
                                       tag="A")
                        xd_t = gb.tile([128, (GCALL // 128) * D], bf16,
                                       tag="B")
                        nc.gpsimd.dma_gather(
                            out_ap=xs_t[:, :cc * D].rearrange(
                                "p (c d) -> p c d", d=D),
                            in_ap=banks[ba][:, :],
                            idxs_ap=sidx[:, icol_off:icol_off + ic],
                            num_idxs=cap, num_idxs_reg=cap, elem_size=D,
                            single_packet=False, queue_num=qn % 4,
                        )
                        qn += 1
                        nc.gpsimd.dma_gather(
                            out_ap=xd_t[:, :cc * D].rearrange(
                                "p (c d) -> p c d", d=D),
                            in_ap=banks[bb][:, :],
                            idxs_ap=didx[:, icol_off:icol_off + ic],
                            num_idxs=cap, num_idxs_reg=cap, elem_size=D,
                            single_packet=False, queue_num=qn % 4,
                        )
                        qn += 1
                        nc.vector.tensor_mul(out=xs_t[:, :cc * D],
                                             in0=xs_t[:, :cc * D],
                                             in1=xd_t[:, :cc * D])
                        nc.vector.tensor_reduce(
                            out=score[:, scol_off:scol_off + cc],
                            in_=xs_t[:, :cc * D].rearrange(
                                "p (c d) -> p c d", d=D),
                            axis=mybir.AxisListType.X,
                            op=mybir.AluOpType.add,
                        )
                        icol_off += ic
                        scol_off += cc

                nc.sync.dma_start(out=out_d.ap(), in_=score[:, :])

    nc.compile()
    return nc


def _node_map(n):
    """node id -> (bank, bank-local index) for the quarter-AllGather layout.

    AG chunk q gathers quarter q (3125 rows) of every core's slice; core
    r's quarter lands at rows [r*3125, (r+1)*3125) of htab_q = bank q.
    """
    r = n // SLICE
    rem = n - r * SLICE
    q = rem // QSL
    w = rem - q * QSL
    return q, r * QSL + w


def _wrap_idx(flat):
    """[cap] int16 -> [128, cap//16] in dma_gather's 16-partition wrap."""
    blk = flat.reshape(-1, 16).T  # index i at [i%16, i//16]
    return np.tile(blk, (8, 1))


def _group_edges(src_l, dst_l):
    """Group one core's edges by (src_bank, dst_bank); sort by src id."""
    sb, sl = _node_map(src_l)
    db, dl = _node_map(dst_l)
    key = sb * NBANK + db
    order = np.argsort(key, kind="stable")
    sizes = np.bincount(key, minlength=NGRP)
    groups = {}
    off = 0
    for g in range(NGRP):
        ids = order[off:off + sizes[g]]
        off += sizes[g]
        # ascending src addresses give the src-side gather descriptors
        # HBM locality (the dst side stays random)
        ids = ids[np.argsort(sl[ids], kind="stable")]
        groups[g] = (ids, sl[ids], dl[ids])
    return groups


def _pack_core(groups, caps):
    """Build idx tilings + inverse edge map for one core."""
    icols_total = sum(caps) // 16
    sidx = np.zeros((128, icols_total), dtype=np.int16)
    didx = np.zeros((128, icols_total), dtype=np.int16)
    rows = np.empty(EPC, dtype=np.int64)
    cols = np.empty(EPC, dtype=np.int64)
    icol_off = 0
    scol_off = 0
    for g in GROUP_ORDER:
        ids, sl_g, dl_g = groups[g]
        cap_g = caps[g]
        s_pad = np.zeros(cap_g, dtype=np.int16)
        d_pad = np.zeros(cap_g, dtype=np.int16)
        s_pad[:ids.size] = sl_g
        d_pad[:ids.size] = dl_g
        pos = 0
        for cap in _call_caps(cap_g):
            ic = cap // 16
            seg = slice(pos, pos + cap)
            sidx[:, icol_off:icol_off + ic] = _wrap_idx(s_pad[seg])
            didx[:, icol_off:icol_off + ic] = _wrap_idx(d_pad[seg])
            icol_off += ic
            pos += cap
        j = np.arange(ids.size)
        rows[ids] = j % 128
        cols[ids] = scol_off + j // 128
        scol_off += cap_g // 128
    return sidx, didx, rows, cols


def kernel(x, src, dst):
    global LAST_RESULTS
    from concourse.bass_utils import run_bass_kernel_spmd

    x32 = np.ascontiguousarray(np.asarray(x, dtype=np.float32))
    src_i = np.asarray(src).astype(np.int64)
    dst_i = np.asarray(dst).astype(np.int64)

    core_groups = []
    for i in range(CORES):
        core_groups.append(_group_edges(
            src_i[i * EPC:(i + 1) * EPC], dst_i[i * EPC:(i + 1) * EPC]))

    # JIT-exact per-group capacities: max over cores, rounded up to 128
    caps = []
    for g in range(NGRP):
        m = max(cg[g][0].size for cg in core_groups)
        caps.append(((m + 127) // 128) * 128)
    caps = tuple(caps)

    if caps not in _CACHE:
        _CACHE[caps] = _build(caps)
    nc = _CACHE[caps]

    in_maps = []
    inv = []
    for i in range(CORES):
        sidx, didx, rows, cols = _pack_core(core_groups[i], caps)
        inv.append((rows, cols))
        xs = x32[i * SLICE:(i + 1) * SLICE]
        # [12500,128] -> [4,125,25,128] -> [125, 4,25,128] -> [125, 100*128]
        xsl = np.ascontiguousarray(
            xs.reshape(4, 125, 25, D).transpose(1, 0, 2, 3).reshape(
                125, 100 * D))
        in_maps.append({
            "xsl": xsl,
            "src_idx": np.ascontiguousarray(sidx),
            "dst_idx": np.ascontiguousarray(didx),
        })

    res = run_bass_kernel_spmd(nc, in_maps, core_ids=list(range(CORES)),
                               **RUN_KWARGS)
    LAST_RESULTS = res

    out = np.empty(E, dtype=np.float32)
    for i in range(CORES):
        tilev = np.asarray(res.results[i]["out"])
        rows, cols = inv[i]
        out[i * EPC:(i + 1) * EPC] = tilev[rows, cols]
    return out.reshape(E, 1)


# revision 30
# speedup vs baseline: 1.0695x; 1.0364x over previous
"""Trainium2 Bass kernel for nn_CTRPredictor (gnn_message_passing).

score[e] = dot(normalize(x[src[e]]), normalize(x[dst[e]]))  for E edges.

Strategy (8 NeuronCores, SPMD), v2:
  - Edges sharded: core i gets edges [i*80000, (i+1)*80000).
  - Each core L2-normalizes its 12500-node slice of x in 4 pipelined
    quarter-chunks ([125, 100*128] layout so每 chunk uses 125 partitions),
    emitting each normalized quarter to a quarter AllGather as soon as it
    is ready; AG chunk q produces table bank q (25000 rows, bf16) so
    gathers on early banks overlap later collectives.
  - Host groups each core's edges by (src_bank, dst_bank) into 16 groups
    (bank-local indices fit dma_gather's int16) with JIT-exact capacities
    (max over cores, rounded to 128) instead of a fixed padded capacity.
  - Per group: dma_gather x_norm[src] and x_norm[dst] rows (256B bf16)
    across 4 SWDGE queues, DVE bf16 multiply + grouped reduce -> scores.
  - Host un-permutes scores back to edge order.
"""

import numpy as np

N = 100000
D = 128
E = 640000
CORES = 8
EPC = E // CORES          # 80000 edges per core
SLICE = N // CORES        # 12500 nodes normalized per core
QSL = SLICE // 4          # 3125-row quarter slices (AllGather chunks)
NBANK = 4
BANK = N // NBANK         # 25000 rows per bank (= one AG chunk output)
NGRP = NBANK * NBANK      # 16 (src_bank, dst_bank) groups
GCALL = 2688              # max indices per dma_gather call

# groups in bank-availability order: group (a,b) is gatherable once
# AllGather max(a,b) has completed
GROUP_ORDER = sorted(range(NGRP),
                     key=lambda g: (max(g // NBANK, g % NBANK),
                                    g // NBANK, g % NBANK))

_CACHE = {}
LAST_RESULTS = None
RUN_KWARGS = {}  # extra kwargs for run_bass_kernel_spmd (used by test harness)


def _call_caps(cap):
    """Split a group capacity (multiple of 128) into dma_gather call sizes."""
    calls = []
    while cap > 0:
        c = min(GCALL, cap)
        calls.append(c)
        cap -= c
    return calls


def _build(caps):
    """caps: tuple of 16 per-group capacities (each a multiple of 128)."""
    from concourse import bass, bacc, tile, mybir

    f32 = mybir.dt.float32
    bf16 = mybir.dt.bfloat16
    i16 = mybir.dt.int16

    icols_total = sum(caps) // 16
    scol_total = sum(caps) // 128

    nc = bacc.Bacc("TRN2", target_bir_lowering=False, debug=False,
                   num_devices=CORES, num_swdge_queues=4,
                   dynamic_dma_scratch_size=40960)

    # node slice in [125, 100*128] layout: slice-local node n with
    # q=n//3125, w=n%3125 sits at partition w//25, col q*25 + w%25.
    xsl_d = nc.dram_tensor("xsl", [125, 100 * D], f32, kind="ExternalInput")
    sidx_d = nc.dram_tensor("src_idx", [128, icols_total], i16,
                            kind="ExternalInput")
    didx_d = nc.dram_tensor("dst_idx", [128, icols_total], i16,
                            kind="ExternalInput")
    out_d = nc.dram_tensor("out", [128, scol_total], f32,
                           kind="ExternalOutput")

    CCH = 25 * D  # free-dim span of one quarter chunk [125, 25*128]

    with tile.TileContext(nc) as tc:
        with tc.tile_pool(name="dram", bufs=1, space="DRAM") as dp, \
             tc.tile_pool(name="persist", bufs=1) as pp:

            # ---- index tables + score accumulator ----
            sidx = pp.tile([128, icols_total], i16)
            didx = pp.tile([128, icols_total], i16)
            nc.sync.dma_start(out=sidx[:, :], in_=sidx_d.ap())
            nc.sync.dma_start(out=didx[:, :], in_=didx_d.ap())
            score = pp.tile([128, scol_total], f32)

            # ---- phase 0: normalize this core's slice in 4 chunks ----
            banks = []
            with tc.tile_pool(name="ph0", bufs=1) as p0, \
                 tc.tile_pool(name="sqp", bufs=2) as sqp:
                xsl = p0.tile([125, 100 * D], f32)
                ntile = p0.tile([125, 100 * D], bf16)
                ns = p0.tile([125, 100], f32)
                nrm = p0.tile([125, 100], f32)
                rns = p0.tile([125, 100], f32)
                for q in range(4):
                    csl = slice(q * CCH, (q + 1) * CCH)
                    nsl = slice(q * 25, (q + 1) * 25)
                    nc.sync.dma_start(out=xsl[:, csl],
                                      in_=xsl_d.ap()[:, csl])
                    sq = sqp.tile([125, CCH], f32, tag="sq")
                    nc.scalar.activation(
                        out=sq[:, :], in_=xsl[:, csl],
                        func=mybir.ActivationFunctionType.Square)
                    nc.vector.tensor_reduce(
                        out=ns[:, nsl],
                        in_=sq[:, :].rearrange("p (r d) -> p r d", d=D),
                        axis=mybir.AxisListType.X,
                        op=mybir.AluOpType.add,
                    )
                    nc.scalar.activation(
                        out=nrm[:, nsl], in_=ns[:, nsl],
                        func=mybir.ActivationFunctionType.Sqrt)
                    nc.vector.reciprocal(out=rns[:, nsl], in_=nrm[:, nsl])
                    nc.vector.tensor_mul(
                        out=ntile[:, csl].rearrange("p (r d) -> p r d", d=D),
                        in0=xsl[:, csl].rearrange("p (r d) -> p r d", d=D),
                        in1=rns[:, nsl].unsqueeze(-1).to_broadcast(
                            [125, 25, D]),
                    )
                    # quarter AllGather: output is table bank q
                    agin = dp.tile([QSL, D], bf16, name=f"agin{q}")
                    htab = dp.tile([BANK, D], bf16, name=f"htab{q}",
                                   addr_space="Shared")
                    nc.sync.dma_start(
                        out=agin[:, :].rearrange("(p r) d -> p (r d)", p=125),
                        in_=ntile[:, csl],
                    )
                    nc.gpsimd.collective_compute(
                        "AllGather",
                        mybir.AluOpType.bypass,
                        replica_groups=[list(range(CORES))],
                        ins=[agin.opt()],
                        outs=[htab.opt()],
                    )
                    banks.append(htab)

            # ---- main loop: gathers on 4 queues, DVE dot per call ----
            with tc.tile_pool(name="ga", bufs=5) as ga, \
                 tc.tile_pool(name="gb", bufs=5) as gb:
                qn = 0
                icol_off = 0
                scol_off = 0
                for g in GROUP_ORDER:
                    ba, bb = g // NBANK, g % NBANK
                    for cap in _call_caps(caps[g]):
                        cc = cap // 128   # gathered row-columns this call
                        ic = cap // 16    # index columns this call
                        xs_t = ga.tile([128, (GCALL // 128) * D], bf16,
                                       tag="A")
                        xd_t = gb.tile([128, (GCALL // 128) * D], bf16,
                                       tag="B")
                        nc.gpsimd.dma_gather(
                            out_ap=xs_t[:, :cc * D].rearrange(
                                "p (c d) -> p c d", d=D),
                            in_ap=banks[ba][:, :],
                            idxs_ap=sidx[:, icol_off:icol_off + ic],
                            num_idxs=cap, num_idxs_reg=cap, elem_size=D,
                            single_packet=False, queue_num=qn % 4,
                        )
                        qn += 1
                        nc.gpsimd.dma_gather(
                            out_ap=xd_t[:, :cc * D].rearrange(
                                "p (c d) -> p c d", d=D),
                            in_ap=banks[bb][:, :],
                            idxs_ap=didx[:, icol_off:icol_off + ic],
                            num_idxs=cap, num_idxs_reg=cap, elem_size=D,
                            single_packet=False, queue_num=qn % 4,
                        )
                        qn += 1
                        nc.vector.tensor_mul(out=xs_t[:, :cc * D],
                                             in0=xs_t[:, :cc * D],
                                             in1=xd_t[:, :cc * D])
                        nc.vector.tensor_reduce(
                            out=score[:, scol_off:scol_off + cc],
                            in_=xs_t[:, :cc * D].rearrange(
                                "p (c d) -> p c d", d=D),
                            axis=mybir.AxisListType.X,
                            op=mybir.AluOpType.add,
                        )
                        icol_off += ic
                        scol_off += cc

                nc.sync.dma_start(out=out_d.ap(), in_=score[:, :])

    nc.compile()
    return nc


def _node_map(n):
    """node id -> (bank, bank-local index) for the quarter-AllGather layout.

    AG chunk q gathers quarter q (3125 rows) of every core's slice; core
    r's quarter lands at rows [r*3125, (r+1)*3125) of htab_q = bank q.
    """
    r = n // SLICE
    rem = n - r * SLICE
    q = rem // QSL
    w = rem - q * QSL
    return q, r * QSL + w


def _wrap_idx(flat):
    """[cap] int16 -> [128, cap//16] in dma_gather's 16-partition wrap."""
    blk = flat.reshape(-1, 16).T  # index i at [i%16, i//16]
    return np.tile(blk, (8, 1))


def _group_edges(src_l, dst_l):
    """Group one core's edges by (src_bank, dst_bank); sort by src id."""
    sb, sl = _node_map(src_l)
    db, dl = _node_map(dst_l)
    key = sb * NBANK + db
    order = np.argsort(key, kind="stable")
    sizes = np.bincount(key, minlength=NGRP)
    groups = {}
    off = 0
    for g in range(NGRP):
        ids = order[off:off + sizes[g]]
        off += sizes[g]
        # ascending src addresses give the src-side gather descriptors
        # HBM locality (the dst side stays random)
        ids = ids[np.argsort(sl[ids], kind="stable")]
        groups[g] = (ids, sl[ids], dl[ids])
    return groups


def _pack_core(groups, caps):
    """Build idx tilings + inverse edge map for one core."""
    icols_total = sum(caps) // 16
    sidx = np.zeros((128, icols_total), dtype=np.int16)
    didx = np.zeros((128, icols_total), dtype=np.int16)
    rows = np.empty(EPC, dtype=np.int64)
    cols = np.empty(EPC, dtype=np.int64)
    icol_off = 0
    scol_off = 0
    for g in GROUP_ORDER:
        ids, sl_g, dl_g = groups[g]
        cap_g = caps[g]
        s_pad = np.zeros(cap_g, dtype=np.int16)
        d_pad = np.zeros(cap_g, dtype=np.int16)
        s_pad[:ids.size] = sl_g
        d_pad[:ids.size] = dl_g
        pos = 0
        for cap in _call_caps(cap_g):
            ic = cap // 16
            seg = slice(pos, pos + cap)
            sidx[:, icol_off:icol_off + ic] = _wrap_idx(s_pad[seg])
            didx[:, icol_off:icol_off + ic] = _wrap_idx(d_pad[seg])
            icol_off += ic
            pos += cap
        j = np.arange(ids.size)
        rows[ids] = j % 128
        cols[ids] = scol_off + j // 128
        scol_off += cap_g // 128
    return sidx, didx, rows, cols


def kernel(x, src, dst):
    global LAST_RESULTS
    from concourse.bass_utils import run_bass_kernel_spmd

    x32 = np.ascontiguousarray(np.asarray(x, dtype=np.float32))
    src_i = np.asarray(src).astype(np.int64)
    dst_i = np.asarray(dst).astype(np.int64)

    core_groups = []
    for i in range(CORES):
        core_groups.append(_group_edges(
            src_i[i * EPC:(i + 1) * EPC], dst_i[i * EPC:(i + 1) * EPC]))

    # JIT-exact per-group capacities: max over cores, rounded up to 128
    caps = []
    for g in range(NGRP):
        m = max(cg[g][0].size for cg in core_groups)
        caps.append(((m + 127) // 128) * 128)
    caps = tuple(caps)

    if caps not in _CACHE:
        _CACHE[caps] = _build(caps)
    nc = _CACHE[caps]

    in_maps = []
    inv = []
    for i in range(CORES):
        sidx, didx, rows, cols = _pack_core(core_groups[i], caps)
        inv.append((rows, cols))
        xs = x32[i * SLICE:(i + 1) * SLICE]
        # [12500,128] -> [4,125,25,128] -> [125, 4,25,128] -> [125, 100*128]
        xsl = np.ascontiguousarray(
            xs.reshape(4, 125, 25, D).transpose(1, 0, 2, 3).reshape(
                125, 100 * D))
        in_maps.append({
            "xsl": xsl,
            "src_idx": np.ascontiguousarray(sidx),
            "dst_idx": np.ascontiguousarray(didx),
        })

    res = run_bass_kernel_spmd(nc, in_maps, core_ids=list(range(CORES)),
                               **RUN_KWARGS)
    LAST_RESULTS = res

    out = np.empty(E, dtype=np.float32)
    for i in range(CORES):
        tilev = np.asarray(res.results[i]["out"])
        rows, cols = inv[i]
        out[i * EPC:(i + 1) * EPC] = tilev[rows, cols]
    return out.reshape(E, 1)
